# revision 27
# baseline (speedup 1.0000x reference)
"""Trainium2 Bass kernel for nn_BertAttentionDistance (B=4, S=2048, H=1024, NH=1, DT=32).

Sharding: 8 cores = (batch b = c//2) x (query-half qh = c%2, 1024 q-rows each).
K/V projection work for a batch is replicated across its 2 cores (no collectives).

Math notes (exact identities w.r.t. the reference):
  * take_along_axis(word_dot_distance, rel, 3) * (rel == 1)
      == (q . dist_emb[1]) * (rel == 1)           (gather collapses)
  * softmax max-subtraction skipped: scores/32 is O(+-3), safely in fp32 exp range.
  * attention_mask enters as per-k additive bias inside exp():
      exp((s + dist)/32 + am_k)  -- the ACT per-partition bias slot (k on partitions).
  * v-bias and o-bias fold into the residual: x = Wo@ctx + (Wo@bv + bo) + hs,
      folded on host into hsq' = hs_q + Wo@bv + bo.
  * q/k biases applied on the PSUM->SBUF evacuation (per-partition tensor_scalar add).
  * softmax 1/denominator folded into the ctxT PSUM->SBUF evacuation multiply.

Dataflow (per core, all matmul inputs bf16, fp32 PSUM accumulate):
  v[s,d]   = hsT^T-chunks @ WvT         (lhsT=hsT chunk, rhs=WvT)   [s-part, d-free]
  kT[h,k]  = (Wk^T)^T-chunks @ hsT      (lhsT=WkT chunk, rhs=hsT)   [d-part, k-free]
  qT[h,q]  likewise from hsqT (this core's query rows)
  ST[k,q]  = kT^T @ qT                  (lhsT=kT chunk,  rhs=qT)    transposed scores
  expT     = exp((ST + maskT*wdd1)/32 + am_k)    maskT = (relT == 1)
  denom[q] = ones^T @ expT (PE row-sum) -> gpsimd partition_broadcast -> 1/denom
  ctxT[d,q]= (v^T-chunks @ expT) * (1/denom)[q]  (lhsT=v chunk, rhs=expT)
  attn[s,h]= ctxT^T-chunks @ WoT        (lhsT=ctxT chunk, rhs=WoT)
  x        = attn + hsq'                then LayerNorm over h.
The epilogue (out-proj + LN) for query-chunk 0 is emitted right after its PV so
it overlaps the PE shadow of query-chunk 1's scores/PV.
"""

import sys

for p in ("/opt/trn_rl_repo", "/opt/pypackages"):
    if p not in sys.path:
        sys.path.insert(0, p)

from contextlib import ExitStack

import ml_dtypes
import numpy as np

import concourse.bacc as bacc
import concourse.bass as bass
import concourse.tile as tile
from concourse import mybir
from concourse.bass_utils import run_bass_kernel_spmd

# Problem constants (hardcoded per the harness contract).
B, S, H = 4, 2048, 1024
N_CORES = 8
SQ = 1024          # query rows per core
HC = H // 128      # 8 chunks of 128 over hidden/head dim
KC = S // 128      # 16 chunks of 128 over key dim
QN = SQ // 512     # 2 chunks of 512 over this core's query rows
LN_EPS = 1e-12
INV_SQRT_DH = 1.0 / 32.0

F32 = mybir.dt.float32
BF16 = mybir.dt.bfloat16
I8 = mybir.dt.int8

_CACHE = {}


def _build_program(ln_affine=True):
    nc = bacc.Bacc("TRN2", target_bir_lowering=False, debug=False)

    hsT = nc.dram_tensor("hsT", [H, S], BF16, kind="ExternalInput")
    hsqT = nc.dram_tensor("hsqT", [H, SQ], BF16, kind="ExternalInput")
    hsq = nc.dram_tensor("hsq", [SQ, H], F32, kind="ExternalInput")
    relT = nc.dram_tensor("relT", [S, SQ], I8, kind="ExternalInput")
    wqT = nc.dram_tensor("wqT", [H, H], BF16, kind="ExternalInput")
    wkT = nc.dram_tensor("wkT", [H, H], BF16, kind="ExternalInput")
    wvT = nc.dram_tensor("wvT", [H, H], BF16, kind="ExternalInput")
    woT = nc.dram_tensor("woT", [H, H], BF16, kind="ExternalInput")
    bq_d = nc.dram_tensor("bq", [128, HC], F32, kind="ExternalInput")
    bk_d = nc.dram_tensor("bk", [128, HC], F32, kind="ExternalInput")
    d1_d = nc.dram_tensor("d1", [128, HC], BF16, kind="ExternalInput")
    am_d = nc.dram_tensor("am", [128, KC], F32, kind="ExternalInput")
    lng_d = nc.dram_tensor("lng", [H], F32, kind="ExternalInput")
    lnb_d = nc.dram_tensor("lnb", [H], F32, kind="ExternalInput")
    out_d = nc.dram_tensor("out", [SQ, H], F32, kind="ExternalOutput")

    def bcast_rows(src_1d_ap, p=128):
        """AP that reads a 1-D DRAM row broadcast across p partitions."""
        return bass.AP(
            tensor=src_1d_ap.tensor,
            offset=src_1d_ap.offset,
            ap=[[0, p], *src_1d_ap.ap],
        )

    with tile.TileContext(nc) as tc, ExitStack() as ctx:
        consts = ctx.enter_context(tc.tile_pool(name="consts", bufs=1))
        big = ctx.enter_context(tc.tile_pool(name="big", bufs=1))
        psum_sm = ctx.enter_context(tc.tile_pool(name="psum_sm", bufs=2, space="PSUM"))

        # ---- constants ----
        ones_t = consts.tile([128, 1], BF16)
        nc.vector.memset(ones_t, 1.0)
        eps_t = consts.tile([128, 1], F32)
        nc.vector.memset(eps_t, LN_EPS)
        bq_t = consts.tile([128, HC], F32)
        nc.sync.dma_start(bq_t, bq_d[:])
        bk_t = consts.tile([128, HC], F32)
        nc.sync.dma_start(bk_t, bk_d[:])
        d1_t = consts.tile([128, HC], BF16)
        nc.sync.dma_start(d1_t, d1_d[:])
        am_t = consts.tile([128, KC], F32)
        nc.sync.dma_start(am_t, am_d[:])
        g_b = consts.tile([128, H], F32)
        nc.gpsimd.dma_start(g_b, bcast_rows(lng_d[:]))
        b_b = consts.tile([128, H], F32)
        nc.gpsimd.dma_start(b_b, bcast_rows(lnb_d[:]))

        # ---- persistent big tensors ----
        kT = big.tile([128, HC, S], BF16)       # [d-part, dc, k]
        v_sb = big.tile([128, KC, H], BF16)     # [s-part, kc, d]
        ctxT = big.tile([128, HC, SQ], BF16)    # [d-part, dc, q] (normalized)

        with tc.tile_pool(name="qpool", bufs=1) as qpool:
            qT = qpool.tile([128, HC, SQ], BF16)    # [d-part, dc, q]
            wdd1_b = qpool.tile([128, SQ], F32)     # broadcast of q . dist_emb[1]
            wdd1_row = qpool.tile([1, SQ], F32)

            # ================= phase 1: projections =================
            with (
                tc.tile_pool(name="hsT_pool", bufs=1) as hsT_pool,
                tc.tile_pool(name="wpool", bufs=2) as wpool,
                tc.tile_pool(name="psum_p", bufs=4, space="PSUM") as psum_p,
            ):
                # DMA order paced to v-projection consumption: wv half 0 +
                # hsT column block 0 first (first matmul after ~2MB), the
                # rest interleaved.
                wv_sb = wpool.tile([128, HC, H], BF16, tag="w")
                wv_r = wvT[:].rearrange("(c p) m -> p c m", p=128)
                hsT_sb = hsT_pool.tile([128, HC, S], BF16)
                hsT_r = hsT[:].rearrange("(c p) s -> p c s", p=128)

                def dma_hsT_block(nb):
                    nc.sync.dma_start(
                        hsT_sb[:, :, nb * 512:(nb + 1) * 512],
                        hsT_r[:, :, nb * 512:(nb + 1) * 512],
                    )

                nc.sync.dma_start(wv_sb[:, :, 0:512], wv_r[:, :, 0:512])
                dma_hsT_block(0)
                dma_hsT_block(1)
                nc.sync.dma_start(wv_sb[:, :, 512:1024], wv_r[:, :, 512:1024])
                dma_hsT_block(2)
                dma_hsT_block(3)

                # v[s,d]: lhsT = hsT chunk (stationary), rhs = WvT columns
                for n0 in range(0, H, 512):
                    for sc in range(KC):
                        ps = psum_p.tile([128, 512], F32, tag="pp")
                        for dc in range(HC):
                            nc.tensor.matmul(
                                ps,
                                hsT_sb[:, dc, sc * 128:(sc + 1) * 128],
                                wv_sb[:, dc, n0:n0 + 512],
                                start=(dc == 0),
                                stop=(dc == HC - 1),
                            )
                        nc.any.tensor_copy(v_sb[:, sc, n0:n0 + 512], ps)

                # k then q projections: dst[h=mc, col n] += W^T-chunk @ src
                for w_d, bias_t, dst, src, ncols in (
                    (wkT, bk_t, kT, hsT_sb, S),
                    (wqT, bq_t, qT, None, SQ),
                ):
                    w_sb = wpool.tile([128, HC, H], BF16, tag="w")
                    w_r = w_d[:].rearrange("(c p) m -> p c m", p=128)
                    for mc in range(HC):
                        nc.sync.dma_start(
                            w_sb[:, :, mc * 128:(mc + 1) * 128],
                            w_r[:, :, mc * 128:(mc + 1) * 128],
                        )
                    if src is None:  # q: DMA this core's query rows
                        src = hsT_pool.tile([128, HC, SQ], BF16)
                        hsqT_r = hsqT[:].rearrange("(c p) s -> p c s", p=128)
                        for nb in range(2):
                            nc.sync.dma_start(
                                src[:, :, nb * 512:(nb + 1) * 512],
                                hsqT_r[:, :, nb * 512:(nb + 1) * 512],
                            )
                    for n0 in range(0, ncols, 512):
                        for mc in range(HC):
                            ps = psum_p.tile([128, 512], F32, tag="pp")
                            for dc in range(HC):
                                nc.tensor.matmul(
                                    ps,
                                    w_sb[:, dc, mc * 128:(mc + 1) * 128],
                                    src[:, dc, n0:n0 + 512],
                                    start=(dc == 0),
                                    stop=(dc == HC - 1),
                                )
                            nc.any.tensor_scalar_add(
                                dst[:, mc, n0:n0 + 512], ps, bias_t[:, mc:mc + 1]
                            )

                # wdd1[q] = q . dist_emb[1] (M=1 matmuls), partition-broadcast
                for qn in range(QN):
                    q_sl = slice(qn * 512, (qn + 1) * 512)
                    ps1 = psum_sm.tile([1, 512], F32, tag="small")
                    for dc in range(HC):
                        nc.tensor.matmul(
                            ps1,
                            d1_t[:, dc:dc + 1],
                            qT[:, dc, q_sl],
                            start=(dc == 0),
                            stop=(dc == HC - 1),
                        )
                    nc.any.tensor_copy(wdd1_row[:, q_sl], ps1)
                nc.gpsimd.partition_broadcast(wdd1_b, wdd1_row)

            # ====== phase 2+3 interleaved per query-chunk of 512 ======
            with (
                tc.tile_pool(name="expp", bufs=2) as expp,
                tc.tile_pool(name="relp", bufs=3) as relp,
                tc.tile_pool(name="smx", bufs=3) as smx,
                tc.tile_pool(name="wo_pool", bufs=1) as wo_pool,
                tc.tile_pool(name="epi", bufs=2) as epi,
                tc.tile_pool(name="stat", bufs=4) as stat,
                tc.tile_pool(name="psum_s", bufs=3, space="PSUM") as psum_s,
                tc.tile_pool(name="psum_v", bufs=3, space="PSUM") as psum_v,
            ):
                wo_sb = wo_pool.tile([128, HC, H], BF16)
                wo_r = woT[:].rearrange("(c p) m -> p c m", p=128)
                for dc in range(HC):
                    nc.sync.dma_start(wo_sb[:, dc], wo_r[:, dc])

                def scores_phase(qn):
                    # Emits scores matmuls + softmax numerators AND the
                    # denominator row-sum matmuls, lagged one kc so the PE
                    # never waits on the exp chain.
                    q_sl = slice(qn * 512, (qn + 1) * 512)
                    expT = expp.tile([128, KC, 512], BF16, tag="expT")
                    dn = psum_sm.tile([1, 512], F32, tag="small")

                    def den_mm(kc):
                        nc.tensor.matmul(
                            dn,
                            ones_t,
                            expT[:, kc, :],
                            start=(kc == 0),
                            stop=(kc == KC - 1),
                        )

                    for kc in range(KC):
                        ps = psum_s.tile([128, 512], F32, tag="ps")
                        for dc in range(HC):
                            nc.tensor.matmul(
                                ps,
                                kT[:, dc, kc * 128:(kc + 1) * 128],
                                qT[:, dc, q_sl],
                                start=(dc == 0),
                                stop=(dc == HC - 1),
                            )
                        if kc >= 3:
                            den_mm(kc - 3)
                        rel_t = relp.tile([128, 512], I8, tag="rel")
                        nc.sync.dma_start(
                            rel_t, relT[kc * 128:(kc + 1) * 128, q_sl]
                        )
                        y_t = smx.tile([128, 512], F32, tag="y")
                        nc.any.tensor_scalar(
                            y_t, rel_t, 1.0, None, mybir.AluOpType.is_equal
                        )
                        nc.any.tensor_mul(y_t, y_t, wdd1_b[:, q_sl])
                        nc.any.tensor_add(y_t, ps, y_t)
                        # exp((qk + dist)/32 + attention_mask[k])
                        nc.scalar.activation(
                            expT[:, kc, :],
                            y_t,
                            mybir.ActivationFunctionType.Exp,
                            bias=am_t[:, kc:kc + 1],
                            scale=INV_SQRT_DH,
                        )
                    for kc in range(KC - 3, KC):
                        den_mm(kc)
                    dr = smx.tile([1, 512], F32, tag="dr")
                    nc.any.tensor_copy(dr, dn)
                    db = smx.tile([128, 512], F32, tag="db")
                    nc.gpsimd.partition_broadcast(db, dr)
                    rb = smx.tile([128, 512], F32, tag="rb")
                    nc.vector.reciprocal(rb, db)
                    return expT, rb

                def pv_phase(qn, expT, rb):
                    # PV: ctxT[d, q], normalized on evacuation
                    q_sl = slice(qn * 512, (qn + 1) * 512)
                    for dc in range(HC):
                        pv = psum_v.tile([128, 512], F32, tag="pv")
                        for kc in range(KC):
                            nc.tensor.matmul(
                                pv,
                                v_sb[:, kc, dc * 128:(dc + 1) * 128],
                                expT[:, kc, :],
                                start=(kc == 0),
                                stop=(kc == KC - 1),
                            )
                        nc.any.tensor_mul(ctxT[:, dc, q_sl], pv, rb)

                def epilogue(qn):
                    # out-proj + residual + LN for this q-chunk
                    for sc in range(qn * 4, qn * 4 + 4):
                        hsq_t = epi.tile([128, H], F32, tag="hsq")
                        nc.sync.dma_start(hsq_t, hsq[sc * 128:(sc + 1) * 128, :])
                        x_t = epi.tile([128, H], F32, tag="x")
                        for hn in range(2):
                            ao = psum_v.tile([128, 512], F32, tag="pv")
                            for dc in range(HC):
                                nc.tensor.matmul(
                                    ao,
                                    ctxT[:, dc, sc * 128:(sc + 1) * 128],
                                    wo_sb[:, dc, hn * 512:(hn + 1) * 512],
                                    start=(dc == 0),
                                    stop=(dc == HC - 1),
                                )
                            h_sl = slice(hn * 512, (hn + 1) * 512)
                            nc.any.tensor_add(x_t[:, h_sl], ao, hsq_t[:, h_sl])

                        # LayerNorm over h (free dim)
                        st = stat.tile([128, 2, 6], F32, tag="st")
                        nc.vector.bn_stats(st[:, 0, :], x_t[:, 0:512])
                        nc.vector.bn_stats(st[:, 1, :], x_t[:, 512:1024])
                        mv = stat.tile([128, 2], F32, tag="mv")
                        nc.vector.bn_aggr(mv, st)
                        sd = stat.tile([128, 1], F32, tag="sd")
                        nc.scalar.activation(
                            sd, mv[:, 1:2],
                            mybir.ActivationFunctionType.Sqrt, bias=eps_t,
                        )
                        rq = stat.tile([128, 1], F32, tag="rq")
                        nc.vector.reciprocal(rq, sd)
                        y_t = epi.tile([128, H], F32, tag="hsq")
                        nc.vector.tensor_scalar(
                            y_t, x_t, mv[:, 0:1], rq,
                            mybir.AluOpType.subtract, mybir.AluOpType.mult,
                        )
                        if ln_affine:
                            nc.any.tensor_mul(y_t, y_t, g_b)
                            nc.any.tensor_add(y_t, y_t, b_b)
                        nc.sync.dma_start(out_d[sc * 128:(sc + 1) * 128, :], y_t)

                # interleave: epilogue(0) fills the PE shadow between
                # scores(1) and PV(1); den/exp chains hide under matmuls.
                exp0, rb0 = scores_phase(0)
                pv_phase(0, exp0, rb0)
                exp1, rb1 = scores_phase(1)
                epilogue(0)
                pv_phase(1, exp1, rb1)
                epilogue(1)

    nc.compile()
    return nc


def get_program(ln_affine=True):
    key = ("nc", ln_affine)
    if key not in _CACHE:
        _CACHE[key] = _build_program(ln_affine)
    return _CACHE[key]


def make_in_maps(inputs):
    """Host-side sharding / layout prep (numpy only)."""
    f32 = np.float32
    bf16 = ml_dtypes.bfloat16
    hs = np.asarray(inputs["hidden_states"], dtype=f32)
    rel = np.asarray(inputs["word_word_relation"])
    am = np.asarray(inputs["attention_mask"], dtype=f32)  # [B,1,1,S]
    Wq = np.asarray(inputs["Wq"], dtype=f32)
    Wk = np.asarray(inputs["Wk"], dtype=f32)
    Wv = np.asarray(inputs["Wv"], dtype=f32)
    Wo = np.asarray(inputs["Wo"], dtype=f32)
    bq = np.asarray(inputs["bq"], dtype=f32)
    bk = np.asarray(inputs["bk"], dtype=f32)
    bv = np.asarray(inputs["bv"], dtype=f32)
    bo = np.asarray(inputs["bo"], dtype=f32)
    d1 = np.asarray(inputs["dist_emb"], dtype=f32)[1]
    lng = np.asarray(inputs["ln_g"], dtype=f32)
    lnb = np.asarray(inputs["ln_b"], dtype=f32)

    wqT = np.ascontiguousarray(Wq.T).astype(bf16)
    wkT = np.ascontiguousarray(Wk.T).astype(bf16)
    wvT = np.ascontiguousarray(Wv.T).astype(bf16)
    woT = np.ascontiguousarray(Wo.T).astype(bf16)
    bo_eff = Wo @ bv + bo  # v/o biases fold into the residual
    bq_t = np.ascontiguousarray(bq.reshape(HC, 128).T)
    bk_t = np.ascontiguousarray(bk.reshape(HC, 128).T)
    d1_t = np.ascontiguousarray(d1.reshape(HC, 128).T).astype(bf16)
    rel8 = rel.astype(np.int8)

    in_maps = []
    for c in range(N_CORES):
        b, qh = divmod(c, 2)
        qs = qh * SQ
        in_maps.append({
            "hsT": hs[b].T.astype(bf16),
            "hsqT": hs[b, qs:qs + SQ, :].T.astype(bf16),
            "hsq": hs[b, qs:qs + SQ, :] + bo_eff,
            "relT": np.ascontiguousarray(rel8[b, qs:qs + SQ, :].T),
            "wqT": wqT, "wkT": wkT, "wvT": wvT, "woT": woT,
            "bq": bq_t, "bk": bk_t, "d1": d1_t,
            "am": np.ascontiguousarray(am[b, 0, 0].reshape(KC, 128).T),
            "lng": lng, "lnb": lnb,
        })
    return in_maps


def kernel(**inputs):
    ln_affine = not (
        np.all(np.asarray(inputs["ln_g"]) == 1.0)
        and np.all(np.asarray(inputs["ln_b"]) == 0.0)
    )
    nc = get_program(ln_affine)
    in_maps = make_in_maps(inputs)
    res = run_bass_kernel_spmd(nc, in_maps, core_ids=list(range(N_CORES)))
    out = np.empty((B, S, H), dtype=np.float32)
    for c in range(N_CORES):
        b, qh = divmod(c, 2)
        out[b, qh * SQ:(qh + 1) * SQ, :] = res.results[c]["out"]
    return out


# revision 35
# speedup vs baseline: 1.3518x; 1.3518x over previous
"""Trainium2 Bass kernel for nn_BertAttentionDistance (B=4, S=2048, H=1024, NH=1, DT=32).

Sharding: 8 cores = (batch b = c//2) x (query-half qh = c%2, 1024 q-rows each).
K/V projection work for a batch is replicated across its 2 cores (no collectives).

Math notes (exact identities w.r.t. the reference):
  * take_along_axis(word_dot_distance, rel, 3) * (rel == 1)
      == (q . dist_emb[1]) * (rel == 1)           (gather collapses)
  * softmax max-subtraction skipped: scores/32 is O(+-3), safely in fp32 exp range.
  * attention_mask enters as per-k additive bias inside exp():
      exp((s + dist)/32 + am_k)  -- the ACT per-partition bias slot (k on partitions).
  * v-bias and o-bias fold into the residual: x = Wo@ctx + (Wo@bv + bo) + hs,
      folded on host into hsq' = hs_q + Wo@bv + bo.
  * q/k biases applied on the PSUM->SBUF evacuation (per-partition tensor_scalar add).
  * softmax 1/denominator folded into the ctxT PSUM->SBUF evacuation multiply.

Dataflow (per core, all matmul inputs bf16, fp32 PSUM accumulate):
  v[s,d]   = hsT^T-chunks @ WvT         (lhsT=hsT chunk, rhs=WvT)   [s-part, d-free]
  kT[h,k]  = (Wk^T)^T-chunks @ hsT      (lhsT=WkT chunk, rhs=hsT)   [d-part, k-free]
  qT[h,q]  likewise from hsqT (this core's query rows)
  ST[k,q]  = kT^T @ qT                  (lhsT=kT chunk,  rhs=qT)    transposed scores
  expT     = exp((ST + maskT*wdd1)/32 + am_k)    maskT = (relT == 1)
  denom[q] = ones^T @ expT (PE row-sum) -> gpsimd partition_broadcast -> 1/denom
  ctxT[d,q]= (v^T-chunks @ expT) * (1/denom)[q]  (lhsT=v chunk, rhs=expT)
  attn[s,h]= ctxT^T-chunks @ WoT        (lhsT=ctxT chunk, rhs=WoT)
  x        = attn + hsq'                then LayerNorm over h.
The epilogue (out-proj + LN) for query-chunk 0 is emitted right after its PV so
it overlaps the PE shadow of query-chunk 1's scores/PV.
"""

import sys

for p in ("/opt/trn_rl_repo", "/opt/pypackages"):
    if p not in sys.path:
        sys.path.insert(0, p)

from contextlib import ExitStack

import ml_dtypes
import numpy as np

import concourse.bacc as bacc
import concourse.bass as bass
import concourse.tile as tile
from concourse import mybir
from concourse.bass_utils import run_bass_kernel_spmd

# Problem constants (hardcoded per the harness contract).
B, S, H = 4, 2048, 1024
N_CORES = 8
SQ = 1024          # query rows per core
HC = H // 128      # 8 chunks of 128 over hidden/head dim
KC = S // 128      # 16 chunks of 128 over key dim
QN = SQ // 512     # 2 chunks of 512 over this core's query rows
LN_EPS = 1e-12
INV_SQRT_DH = 1.0 / 32.0

F32 = mybir.dt.float32
BF16 = mybir.dt.bfloat16
I8 = mybir.dt.int8
F8 = mybir.dt.float8e4
DR = mybir.MatmulPerfMode.DoubleRow

_CACHE = {}


def _build_program(ln_affine=True):
    nc = bacc.Bacc("TRN2", target_bir_lowering=False, debug=False)

    hsT = nc.dram_tensor("hsT", [H, S], F8, kind="ExternalInput")
    hsqT = nc.dram_tensor("hsqT", [H, SQ], F8, kind="ExternalInput")
    hsq = nc.dram_tensor("hsq", [SQ, H], F32, kind="ExternalInput")
    relT = nc.dram_tensor("relT", [S, SQ], I8, kind="ExternalInput")
    wqT = nc.dram_tensor("wqT", [H, H], F8, kind="ExternalInput")
    wkT = nc.dram_tensor("wkT", [H, H], F8, kind="ExternalInput")
    wvT = nc.dram_tensor("wvT", [H, H], F8, kind="ExternalInput")
    woT = nc.dram_tensor("woT", [H, H], BF16, kind="ExternalInput")
    bq_d = nc.dram_tensor("bq", [128, HC], F32, kind="ExternalInput")
    bk_d = nc.dram_tensor("bk", [128, HC], F32, kind="ExternalInput")
    d1_d = nc.dram_tensor("d1", [128, HC], BF16, kind="ExternalInput")
    am_d = nc.dram_tensor("am", [128, KC], F32, kind="ExternalInput")
    lng_d = nc.dram_tensor("lng", [H], F32, kind="ExternalInput")
    lnb_d = nc.dram_tensor("lnb", [H], F32, kind="ExternalInput")
    out_d = nc.dram_tensor("out", [SQ, H], F32, kind="ExternalOutput")

    def bcast_rows(src_1d_ap, p=128):
        """AP that reads a 1-D DRAM row broadcast across p partitions."""
        return bass.AP(
            tensor=src_1d_ap.tensor,
            offset=src_1d_ap.offset,
            ap=[[0, p], *src_1d_ap.ap],
        )

    with tile.TileContext(nc) as tc, ExitStack() as ctx:
        consts = ctx.enter_context(tc.tile_pool(name="consts", bufs=1))
        big = ctx.enter_context(tc.tile_pool(name="big", bufs=1))
        psum_sm = ctx.enter_context(tc.tile_pool(name="psum_sm", bufs=1, space="PSUM"))

        # ---- constants ----
        ones_t = consts.tile([128, 1], BF16)
        nc.vector.memset(ones_t, 1.0)
        eps_t = consts.tile([128, 1], F32)
        nc.vector.memset(eps_t, LN_EPS)
        bq_t = consts.tile([128, HC], F32)
        nc.sync.dma_start(bq_t, bq_d[:])
        bk_t = consts.tile([128, HC], F32)
        nc.sync.dma_start(bk_t, bk_d[:])
        d1_t = consts.tile([128, HC], BF16)
        nc.sync.dma_start(d1_t, d1_d[:])
        am_t = consts.tile([128, KC], F32)
        nc.sync.dma_start(am_t, am_d[:])
        g_b = consts.tile([128, H], F32)
        nc.gpsimd.dma_start(g_b, bcast_rows(lng_d[:]))
        b_b = consts.tile([128, H], F32)
        nc.gpsimd.dma_start(b_b, bcast_rows(lnb_d[:]))

        # ---- persistent big tensors ----
        kT = big.tile([128, HC, S], BF16)       # [d-part, dc, k]
        v_sb = big.tile([128, KC, H], BF16)     # [s-part, kc, d]
        ctxT = big.tile([128, HC, SQ], BF16)    # [d-part, dc, q] (normalized)

        with tc.tile_pool(name="qpool", bufs=1) as qpool:
            qT = qpool.tile([128, HC, SQ], BF16)    # [d-part, dc, q]
            wdd1_b = qpool.tile([128, SQ], F32)     # broadcast of q . dist_emb[1]
            wdd1_row = qpool.tile([1, SQ], F32)

            # ================= phase 1: projections =================
            with (
                tc.tile_pool(name="hsT_pool", bufs=1) as hsT_pool,
                tc.tile_pool(name="wpool", bufs=2) as wpool,
                tc.tile_pool(name="psum_p", bufs=4, space="PSUM") as psum_p,
            ):
                # DMA order paced to v-projection consumption: wv half 0 +
                # hsT column block 0 first (first matmul after ~2MB), the
                # rest interleaved.
                wv_sb = wpool.tile([128, HC, H], F8, tag="w")
                wv_r = wvT[:].rearrange("(c p) m -> p c m", p=128)
                hsT_sb = hsT_pool.tile([128, HC, S], F8)
                hsT_r = hsT[:].rearrange("(c p) s -> p c s", p=128)

                def dma_hsT_block(nb):
                    nc.sync.dma_start(
                        hsT_sb[:, :, nb * 512:(nb + 1) * 512],
                        hsT_r[:, :, nb * 512:(nb + 1) * 512],
                    )

                nc.sync.dma_start(wv_sb[:, :, 0:512], wv_r[:, :, 0:512])
                dma_hsT_block(0)
                dma_hsT_block(1)
                nc.sync.dma_start(wv_sb[:, :, 512:1024], wv_r[:, :, 512:1024])
                dma_hsT_block(2)
                dma_hsT_block(3)

                # v[s,d]: lhsT = hsT chunk (stationary), rhs = WvT columns
                for n0 in range(0, H, 512):
                    for sc in range(KC):
                        ps = psum_p.tile([128, 512], F32, tag="pp")
                        for dc in range(0, HC, 2):
                            nc.tensor.matmul(
                                ps,
                                hsT_sb[:, dc:dc + 2, sc * 128:(sc + 1) * 128],
                                wv_sb[:, dc:dc + 2, n0:n0 + 512],
                                start=(dc == 0),
                                stop=(dc == HC - 2),
                                perf_mode=DR,
                            )
                        nc.any.tensor_scalar_mul(v_sb[:, sc, n0:n0 + 512], ps, 0.125)

                # k then q projections: dst[h=mc, col n] += W^T-chunk @ src
                for w_d, bias_t, dst, src, ncols in (
                    (wkT, bk_t, kT, hsT_sb, S),
                    (wqT, bq_t, qT, None, SQ),
                ):
                    w_sb = wpool.tile([128, HC, H], F8, tag="w")
                    w_r = w_d[:].rearrange("(c p) m -> p c m", p=128)
                    for mc in range(HC):
                        nc.sync.dma_start(
                            w_sb[:, :, mc * 128:(mc + 1) * 128],
                            w_r[:, :, mc * 128:(mc + 1) * 128],
                        )
                    if src is None:  # q: DMA this core's query rows
                        src = hsT_pool.tile([128, HC, SQ], F8)
                        hsqT_r = hsqT[:].rearrange("(c p) s -> p c s", p=128)
                        for nb in range(2):
                            nc.sync.dma_start(
                                src[:, :, nb * 512:(nb + 1) * 512],
                                hsqT_r[:, :, nb * 512:(nb + 1) * 512],
                            )
                    for n0 in range(0, ncols, 512):
                        for mc in range(HC):
                            ps = psum_p.tile([128, 512], F32, tag="pp")
                            for dc in range(0, HC, 2):
                                nc.tensor.matmul(
                                    ps,
                                    w_sb[:, dc:dc + 2, mc * 128:(mc + 1) * 128],
                                    src[:, dc:dc + 2, n0:n0 + 512],
                                    start=(dc == 0),
                                    stop=(dc == HC - 2),
                                    perf_mode=DR,
                                )
                            nc.any.tensor_scalar(
                                dst[:, mc, n0:n0 + 512], ps,
                                0.125, bias_t[:, mc:mc + 1],
                                mybir.AluOpType.mult, mybir.AluOpType.add,
                            )

                # wdd1[q] = q . dist_emb[1] (M=1 matmuls), partition-broadcast
                for qn in range(QN):
                    q_sl = slice(qn * 512, (qn + 1) * 512)
                    ps1 = psum_sm.tile([1, 512], F32, tag="small")
                    for dc in range(HC):
                        nc.tensor.matmul(
                            ps1,
                            d1_t[:, dc:dc + 1],
                            qT[:, dc, q_sl],
                            start=(dc == 0),
                            stop=(dc == HC - 1),
                        )
                    nc.any.tensor_copy(wdd1_row[:, q_sl], ps1)
                nc.gpsimd.partition_broadcast(wdd1_b, wdd1_row)

            # ====== phase 2+3 interleaved per query-chunk of 512 ======
            with (
                tc.tile_pool(name="expp", bufs=2) as expp,
                tc.tile_pool(name="relp", bufs=2) as relp,
                tc.tile_pool(name="smx", bufs=3) as smx,
                tc.tile_pool(name="wo_pool", bufs=1) as wo_pool,
                tc.tile_pool(name="epi", bufs=3) as epi,
                tc.tile_pool(name="denp", bufs=2) as denp,
                tc.tile_pool(name="stat", bufs=4) as stat,
                tc.tile_pool(name="psum_s", bufs=4, space="PSUM") as psum_s,
                tc.tile_pool(name="psum_v", bufs=3, space="PSUM") as psum_v,
            ):
                wo_sb = wo_pool.tile([128, HC, H], BF16)
                wo_r = woT[:].rearrange("(c p) m -> p c m", p=128)
                for dc in range(HC):
                    nc.sync.dma_start(wo_sb[:, dc], wo_r[:, dc])

                def scores_phase(qn):
                    # Emits scores matmuls + softmax numerators AND the
                    # denominator row-sum matmuls, lagged one kc so the PE
                    # never waits on the exp chain.
                    q_sl = slice(qn * 512, (qn + 1) * 512)
                    expT = expp.tile([128, KC, 512], BF16, tag="expT")
                    dn = psum_sm.tile([1, 512], F32, tag="small")
                    for kc in range(KC):
                        ps = psum_s.tile([128, 512], F32, tag="ps")
                        for dc in range(HC):
                            nc.tensor.matmul(
                                ps,
                                kT[:, dc, kc * 128:(kc + 1) * 128],
                                qT[:, dc, q_sl],
                                start=(dc == 0),
                                stop=(dc == HC - 1),
                            )
                        rel_t = relp.tile([128, 512], I8, tag="rel")
                        nc.sync.dma_start(
                            rel_t, relT[kc * 128:(kc + 1) * 128, q_sl]
                        )
                        y_t = smx.tile([128, 512], F32, tag="y")
                        nc.any.tensor_scalar(
                            y_t, rel_t, 1.0, None, mybir.AluOpType.is_equal
                        )
                        nc.any.tensor_mul(y_t, y_t, wdd1_b[:, q_sl])
                        nc.any.tensor_add(y_t, ps, y_t)
                        # exp((qk + dist)/32 + attention_mask[k])
                        nc.scalar.activation(
                            expT[:, kc, :],
                            y_t,
                            mybir.ActivationFunctionType.Exp,
                            bias=am_t[:, kc:kc + 1],
                            scale=INV_SQRT_DH,
                        )
                    for kc in range(KC):
                        nc.tensor.matmul(
                            dn,
                            ones_t,
                            expT[:, kc, :],
                            start=(kc == 0),
                            stop=(kc == KC - 1),
                        )
                    dr = denp.tile([1, 512], F32, tag="dr")
                    nc.any.tensor_copy(dr, dn)
                    db = denp.tile([128, 512], F32, tag="db")
                    nc.gpsimd.partition_broadcast(db, dr)
                    rb = denp.tile([128, 512], F32, tag="rb")
                    nc.vector.reciprocal(rb, db)
                    return expT, rb

                def pv_phase(qn, expT, rb):
                    # PV: ctxT[d, q], normalized on evacuation
                    q_sl = slice(qn * 512, (qn + 1) * 512)
                    for dc in range(HC):
                        pv = psum_v.tile([128, 512], F32, tag="pv")
                        for kc in range(KC):
                            nc.tensor.matmul(
                                pv,
                                v_sb[:, kc, dc * 128:(dc + 1) * 128],
                                expT[:, kc, :],
                                start=(kc == 0),
                                stop=(kc == KC - 1),
                            )
                        nc.any.tensor_mul(ctxT[:, dc, q_sl], pv, rb)

                def epilogue(qn):
                    # out-proj + residual + LN for this q-chunk
                    for sc in range(qn * 4, qn * 4 + 4):
                        x_t = epi.tile([128, H], F32, tag="x")
                        for hn in range(2):
                            ao = psum_v.tile([128, 512], F32, tag="pv")
                            for dc in range(HC):
                                nc.tensor.matmul(
                                    ao,
                                    ctxT[:, dc, sc * 128:(sc + 1) * 128],
                                    wo_sb[:, dc, hn * 512:(hn + 1) * 512],
                                    start=(dc == 0),
                                    stop=(dc == HC - 1),
                                )
                            h_sl = slice(hn * 512, (hn + 1) * 512)
                            # evacuate on ACT (has slack during the epilogue)
                            nc.scalar.activation(
                                x_t[:, h_sl], ao,
                                mybir.ActivationFunctionType.Copy,
                            )
                            # residual add via accumulating DMA (frees DVE)
                            nc.gpsimd.dma_start(
                                x_t[:, h_sl],
                                hsq[sc * 128:(sc + 1) * 128, h_sl],
                                accum_op=mybir.AluOpType.add,
                            )

                        # LayerNorm over h (free dim)
                        st = stat.tile([128, 2, 6], F32, tag="st")
                        nc.vector.bn_stats(st[:, 0, :], x_t[:, 0:512])
                        nc.vector.bn_stats(st[:, 1, :], x_t[:, 512:1024])
                        mv = stat.tile([128, 2], F32, tag="mv")
                        nc.vector.bn_aggr(mv, st)
                        sd = stat.tile([128, 1], F32, tag="sd")
                        nc.scalar.activation(
                            sd, mv[:, 1:2],
                            mybir.ActivationFunctionType.Sqrt, bias=eps_t,
                        )
                        rq = stat.tile([128, 1], F32, tag="rq")
                        nc.vector.reciprocal(rq, sd)
                        y_t = epi.tile([128, H], F32, tag="hsq")
                        nc.vector.tensor_scalar(
                            y_t, x_t, mv[:, 0:1], rq,
                            mybir.AluOpType.subtract, mybir.AluOpType.mult,
                        )
                        if ln_affine:
                            nc.any.tensor_mul(y_t, y_t, g_b)
                            nc.any.tensor_add(y_t, y_t, b_b)
                        nc.sync.dma_start(out_d[sc * 128:(sc + 1) * 128, :], y_t)

                # interleave: epilogue(0) fills the PE shadow between
                # scores(1) and PV(1); den/exp chains hide under matmuls.
                exp0, rb0 = scores_phase(0)
                pv_phase(0, exp0, rb0)
                exp1, rb1 = scores_phase(1)
                epilogue(0)
                pv_phase(1, exp1, rb1)
                epilogue(1)

    nc.compile()
    return nc


def get_program(ln_affine=True):
    key = ("nc", ln_affine)
    if key not in _CACHE:
        _CACHE[key] = _build_program(ln_affine)
    return _CACHE[key]


def make_in_maps(inputs):
    """Host-side sharding / layout prep (numpy only)."""
    f32 = np.float32
    bf16 = ml_dtypes.bfloat16
    hs = np.asarray(inputs["hidden_states"], dtype=f32)
    rel = np.asarray(inputs["word_word_relation"])
    am = np.asarray(inputs["attention_mask"], dtype=f32)  # [B,1,1,S]
    Wq = np.asarray(inputs["Wq"], dtype=f32)
    Wk = np.asarray(inputs["Wk"], dtype=f32)
    Wv = np.asarray(inputs["Wv"], dtype=f32)
    Wo = np.asarray(inputs["Wo"], dtype=f32)
    bq = np.asarray(inputs["bq"], dtype=f32)
    bk = np.asarray(inputs["bk"], dtype=f32)
    bv = np.asarray(inputs["bv"], dtype=f32)
    bo = np.asarray(inputs["bo"], dtype=f32)
    d1 = np.asarray(inputs["dist_emb"], dtype=f32)[1]
    lng = np.asarray(inputs["ln_g"], dtype=f32)
    lnb = np.asarray(inputs["ln_b"], dtype=f32)

    f8 = ml_dtypes.float8_e4m3
    wqT = np.ascontiguousarray(Wq.T * 8.0).astype(f8)
    wkT = np.ascontiguousarray(Wk.T * 8.0).astype(f8)
    wvT = np.ascontiguousarray(Wv.T * 8.0).astype(f8)
    woT = np.ascontiguousarray(Wo.T).astype(bf16)
    bo_eff = Wo @ bv + bo  # v/o biases fold into the residual
    bq_t = np.ascontiguousarray(bq.reshape(HC, 128).T)
    bk_t = np.ascontiguousarray(bk.reshape(HC, 128).T)
    d1_t = np.ascontiguousarray(d1.reshape(HC, 128).T).astype(bf16)
    rel8 = rel.astype(np.int8)

    in_maps = []
    for c in range(N_CORES):
        b, qh = divmod(c, 2)
        qs = qh * SQ
        in_maps.append({
            "hsT": hs[b].T.astype(f8),
            "hsqT": hs[b, qs:qs + SQ, :].T.astype(f8),
            "hsq": hs[b, qs:qs + SQ, :] + bo_eff,
            "relT": np.ascontiguousarray(rel8[b, qs:qs + SQ, :].T),
            "wqT": wqT, "wkT": wkT, "wvT": wvT, "woT": woT,
            "bq": bq_t, "bk": bk_t, "d1": d1_t,
            "am": np.ascontiguousarray(am[b, 0, 0].reshape(KC, 128).T),
            "lng": lng, "lnb": lnb,
        })
    return in_maps


def kernel(**inputs):
    ln_affine = not (
        np.all(np.asarray(inputs["ln_g"]) == 1.0)
        and np.all(np.asarray(inputs["ln_b"]) == 0.0)
    )
    nc = get_program(ln_affine)
    in_maps = make_in_maps(inputs)
    res = run_bass_kernel_spmd(nc, in_maps, core_ids=list(range(N_CORES)))
    out = np.empty((B, S, H), dtype=np.float32)
    for c in range(N_CORES):
        b, qh = divmod(c, 2)
        out[b, qh * SQ:(qh + 1) * SQ, :] = res.results[c]["out"]
    return out


# revision 37
# speedup vs baseline: 1.5020x; 1.1111x over previous
"""Trainium2 Bass kernel for nn_BertAttentionDistance (B=4, S=2048, H=1024, NH=1, DT=32).

Sharding: 8 cores = (batch b = c//2) x (query-half qh = c%2, 1024 q-rows each).
K/V projection work for a batch is replicated across its 2 cores (no collectives).

Math notes (exact identities w.r.t. the reference):
  * take_along_axis(word_dot_distance, rel, 3) * (rel == 1)
      == (q . dist_emb[1]) * (rel == 1)           (gather collapses)
  * softmax max-subtraction skipped: scores/32 is O(+-3), safely in fp32 exp range.
  * attention_mask enters as per-k additive bias inside exp():
      exp((s + dist)/32 + am_k)  -- the ACT per-partition bias slot (k on partitions).
  * v-bias and o-bias fold into the residual: x = Wo@ctx + (Wo@bv + bo) + hs,
      folded on host into hsq' = hs_q + Wo@bv + bo.
  * q/k biases applied on the PSUM->SBUF evacuation (per-partition tensor_scalar add).
  * softmax 1/denominator folded into the ctxT PSUM->SBUF evacuation multiply.

Dataflow (per core, all matmul inputs bf16, fp32 PSUM accumulate):
  v[s,d]   = hsT^T-chunks @ WvT         (lhsT=hsT chunk, rhs=WvT)   [s-part, d-free]
  kT[h,k]  = (Wk^T)^T-chunks @ hsT      (lhsT=WkT chunk, rhs=hsT)   [d-part, k-free]
  qT[h,q]  likewise from hsqT (this core's query rows)
  ST[k,q]  = kT^T @ qT                  (lhsT=kT chunk,  rhs=qT)    transposed scores
  expT     = exp((ST + maskT*wdd1)/32 + am_k)    maskT = (relT == 1)
  denom[q] = ones^T @ expT (PE row-sum) -> gpsimd partition_broadcast -> 1/denom
  ctxT[d,q]= (v^T-chunks @ expT) * (1/denom)[q]  (lhsT=v chunk, rhs=expT)
  attn[s,h]= ctxT^T-chunks @ WoT        (lhsT=ctxT chunk, rhs=WoT)
  x        = attn + hsq'                then LayerNorm over h.
The epilogue (out-proj + LN) for query-chunk 0 is emitted right after its PV so
it overlaps the PE shadow of query-chunk 1's scores/PV.
"""

import sys

for p in ("/opt/trn_rl_repo", "/opt/pypackages"):
    if p not in sys.path:
        sys.path.insert(0, p)

from contextlib import ExitStack

import ml_dtypes
import numpy as np

import concourse.bacc as bacc
import concourse.bass as bass
import concourse.tile as tile
from concourse import mybir
from concourse.bass_utils import run_bass_kernel_spmd

# Problem constants (hardcoded per the harness contract).
B, S, H = 4, 2048, 1024
N_CORES = 8
SQ = 1024          # query rows per core
HC = H // 128      # 8 chunks of 128 over hidden/head dim
KC = S // 128      # 16 chunks of 128 over key dim
QN = SQ // 512     # 2 chunks of 512 over this core's query rows
LN_EPS = 1e-12
INV_SQRT_DH = 1.0 / 32.0

F32 = mybir.dt.float32
BF16 = mybir.dt.bfloat16
I8 = mybir.dt.int8
F8 = mybir.dt.float8e4
DR = mybir.MatmulPerfMode.DoubleRow

_CACHE = {}


def _build_program(ln_affine=True):
    nc = bacc.Bacc("TRN2", target_bir_lowering=False, debug=False)

    hsT = nc.dram_tensor("hsT", [H, S], F8, kind="ExternalInput")
    hsqT = nc.dram_tensor("hsqT", [H, SQ], F8, kind="ExternalInput")
    hsq = nc.dram_tensor("hsq", [SQ, H], F32, kind="ExternalInput")
    relT = nc.dram_tensor("relT", [S, SQ], I8, kind="ExternalInput")
    wqT = nc.dram_tensor("wqT", [H, H], F8, kind="ExternalInput")
    wkT = nc.dram_tensor("wkT", [H, H], F8, kind="ExternalInput")
    wvT = nc.dram_tensor("wvT", [H, H], F8, kind="ExternalInput")
    woT = nc.dram_tensor("woT", [H, H], BF16, kind="ExternalInput")
    bq_d = nc.dram_tensor("bq", [128, HC], F32, kind="ExternalInput")
    bk_d = nc.dram_tensor("bk", [128, HC], F32, kind="ExternalInput")
    d1_d = nc.dram_tensor("d1", [128, HC], F8, kind="ExternalInput")
    am_d = nc.dram_tensor("am", [128, KC], F32, kind="ExternalInput")
    lng_d = nc.dram_tensor("lng", [H], F32, kind="ExternalInput")
    lnb_d = nc.dram_tensor("lnb", [H], F32, kind="ExternalInput")
    out_d = nc.dram_tensor("out", [SQ, H], F32, kind="ExternalOutput")

    def bcast_rows(src_1d_ap, p=128):
        """AP that reads a 1-D DRAM row broadcast across p partitions."""
        return bass.AP(
            tensor=src_1d_ap.tensor,
            offset=src_1d_ap.offset,
            ap=[[0, p], *src_1d_ap.ap],
        )

    with tile.TileContext(nc) as tc, ExitStack() as ctx:
        consts = ctx.enter_context(tc.tile_pool(name="consts", bufs=1))
        big = ctx.enter_context(tc.tile_pool(name="big", bufs=1))
        psum_sm = ctx.enter_context(tc.tile_pool(name="psum_sm", bufs=1, space="PSUM"))

        # ---- constants ----
        ones_t = consts.tile([128, 1], F8)
        nc.vector.memset(ones_t, 1.0)
        eps_t = consts.tile([128, 1], F32)
        nc.vector.memset(eps_t, LN_EPS)
        bq_t = consts.tile([128, HC], F32)
        nc.sync.dma_start(bq_t, bq_d[:])
        bk_t = consts.tile([128, HC], F32)
        nc.sync.dma_start(bk_t, bk_d[:])
        d1_t = consts.tile([128, HC], F8)
        nc.sync.dma_start(d1_t, d1_d[:])
        am_t = consts.tile([128, KC], F32)
        nc.sync.dma_start(am_t, am_d[:])
        g_b = consts.tile([128, H], F32)
        nc.gpsimd.dma_start(g_b, bcast_rows(lng_d[:]))
        b_b = consts.tile([128, H], F32)
        nc.gpsimd.dma_start(b_b, bcast_rows(lnb_d[:]))

        # ---- persistent big tensors ----
        kT = big.tile([128, HC, S], F8)       # [d-part, dc, k]
        v_sb = big.tile([128, KC, H], F8)     # [s-part, kc, d]
        ctxT = big.tile([128, HC, SQ], BF16)    # [d-part, dc, q] (normalized)

        with tc.tile_pool(name="qpool", bufs=1) as qpool:
            qT = qpool.tile([128, HC, SQ], F8)    # [d-part, dc, q]
            wdd1_b = qpool.tile([128, SQ], F32)     # broadcast of q . dist_emb[1]
            wdd1_row = qpool.tile([1, SQ], F32)

            # ================= phase 1: projections =================
            with (
                tc.tile_pool(name="hsT_pool", bufs=1) as hsT_pool,
                tc.tile_pool(name="wpool", bufs=2) as wpool,
                tc.tile_pool(name="psum_p", bufs=4, space="PSUM") as psum_p,
            ):
                # DMA order paced to v-projection consumption: wv half 0 +
                # hsT column block 0 first (first matmul after ~2MB), the
                # rest interleaved.
                wv_sb = wpool.tile([128, HC, H], F8, tag="w")
                wv_r = wvT[:].rearrange("(c p) m -> p c m", p=128)
                hsT_sb = hsT_pool.tile([128, HC, S], F8)
                hsT_r = hsT[:].rearrange("(c p) s -> p c s", p=128)

                def dma_hsT_block(nb):
                    nc.sync.dma_start(
                        hsT_sb[:, :, nb * 512:(nb + 1) * 512],
                        hsT_r[:, :, nb * 512:(nb + 1) * 512],
                    )

                nc.sync.dma_start(wv_sb[:, :, 0:512], wv_r[:, :, 0:512])
                dma_hsT_block(0)
                dma_hsT_block(1)
                nc.sync.dma_start(wv_sb[:, :, 512:1024], wv_r[:, :, 512:1024])
                dma_hsT_block(2)
                dma_hsT_block(3)

                # v[s,d]: lhsT = hsT chunk (stationary), rhs = WvT columns
                for n0 in range(0, H, 512):
                    for sc in range(KC):
                        ps = psum_p.tile([128, 512], F32, tag="pp")
                        for dc in range(0, HC, 2):
                            nc.tensor.matmul(
                                ps,
                                hsT_sb[:, dc:dc + 2, sc * 128:(sc + 1) * 128],
                                wv_sb[:, dc:dc + 2, n0:n0 + 512],
                                start=(dc == 0),
                                stop=(dc == HC - 2),
                                perf_mode=DR,
                            )
                        nc.any.tensor_scalar_mul(v_sb[:, sc, n0:n0 + 512], ps, 0.125)

                # k then q projections: dst[h=mc, col n] += W^T-chunk @ src
                for w_d, bias_t, dst, src, ncols in (
                    (wkT, bk_t, kT, hsT_sb, S),
                    (wqT, bq_t, qT, None, SQ),
                ):
                    w_sb = wpool.tile([128, HC, H], F8, tag="w")
                    w_r = w_d[:].rearrange("(c p) m -> p c m", p=128)
                    for mc in range(HC):
                        nc.sync.dma_start(
                            w_sb[:, :, mc * 128:(mc + 1) * 128],
                            w_r[:, :, mc * 128:(mc + 1) * 128],
                        )
                    if src is None:  # q: DMA this core's query rows
                        src = hsT_pool.tile([128, HC, SQ], F8)
                        hsqT_r = hsqT[:].rearrange("(c p) s -> p c s", p=128)
                        for nb in range(2):
                            nc.sync.dma_start(
                                src[:, :, nb * 512:(nb + 1) * 512],
                                hsqT_r[:, :, nb * 512:(nb + 1) * 512],
                            )
                    for n0 in range(0, ncols, 512):
                        for mc in range(HC):
                            ps = psum_p.tile([128, 512], F32, tag="pp")
                            for dc in range(0, HC, 2):
                                nc.tensor.matmul(
                                    ps,
                                    w_sb[:, dc:dc + 2, mc * 128:(mc + 1) * 128],
                                    src[:, dc:dc + 2, n0:n0 + 512],
                                    start=(dc == 0),
                                    stop=(dc == HC - 2),
                                    perf_mode=DR,
                                )
                            nc.any.tensor_scalar(
                                dst[:, mc, n0:n0 + 512], ps,
                                0.125, bias_t[:, mc:mc + 1],
                                mybir.AluOpType.mult, mybir.AluOpType.add,
                            )

                # wdd1[q] = q . dist_emb[1] (M=1 matmuls), partition-broadcast
                for qn in range(QN):
                    q_sl = slice(qn * 512, (qn + 1) * 512)
                    ps1 = psum_sm.tile([1, 512], F32, tag="small")
                    for dc in range(HC):
                        nc.tensor.matmul(
                            ps1,
                            d1_t[:, dc:dc + 1],
                            qT[:, dc, q_sl],
                            start=(dc == 0),
                            stop=(dc == HC - 1),
                        )
                    nc.any.tensor_scalar_mul(wdd1_row[:, q_sl], ps1, 0.125)
                nc.gpsimd.partition_broadcast(wdd1_b, wdd1_row)

            # ====== phase 2+3 interleaved per query-chunk of 512 ======
            with (
                tc.tile_pool(name="expp", bufs=2) as expp,
                tc.tile_pool(name="relp", bufs=2) as relp,
                tc.tile_pool(name="smx", bufs=3) as smx,
                tc.tile_pool(name="wo_pool", bufs=1) as wo_pool,
                tc.tile_pool(name="epi", bufs=3) as epi,
                tc.tile_pool(name="denp", bufs=2) as denp,
                tc.tile_pool(name="stat", bufs=4) as stat,
                tc.tile_pool(name="psum_s", bufs=4, space="PSUM") as psum_s,
                tc.tile_pool(name="psum_v", bufs=3, space="PSUM") as psum_v,
            ):
                wo_sb = wo_pool.tile([128, HC, H], BF16)
                wo_r = woT[:].rearrange("(c p) m -> p c m", p=128)
                for dc in range(HC):
                    nc.sync.dma_start(wo_sb[:, dc], wo_r[:, dc])

                def scores_phase(qn):
                    # Emits scores matmuls + softmax numerators AND the
                    # denominator row-sum matmuls, lagged one kc so the PE
                    # never waits on the exp chain.
                    q_sl = slice(qn * 512, (qn + 1) * 512)
                    expT = expp.tile([128, KC, 512], F8, tag="expT")
                    dn = psum_sm.tile([1, 512], F32, tag="small")
                    for kc in range(KC):
                        ps = psum_s.tile([128, 512], F32, tag="ps")
                        for dc in range(0, HC, 2):
                            nc.tensor.matmul(
                                ps,
                                kT[:, dc:dc + 2, kc * 128:(kc + 1) * 128],
                                qT[:, dc:dc + 2, q_sl],
                                start=(dc == 0),
                                stop=(dc == HC - 2),
                                perf_mode=DR,
                            )
                        rel_t = relp.tile([128, 512], I8, tag="rel")
                        nc.sync.dma_start(
                            rel_t, relT[kc * 128:(kc + 1) * 128, q_sl]
                        )
                        y_t = smx.tile([128, 512], F32, tag="y")
                        nc.any.tensor_scalar(
                            y_t, rel_t, 1.0, None, mybir.AluOpType.is_equal
                        )
                        nc.any.tensor_mul(y_t, y_t, wdd1_b[:, q_sl])
                        nc.any.tensor_add(y_t, ps, y_t)
                        # exp((qk + dist)/32 + attention_mask[k])
                        nc.scalar.activation(
                            expT[:, kc, :],
                            y_t,
                            mybir.ActivationFunctionType.Exp,
                            bias=am_t[:, kc:kc + 1],
                            scale=INV_SQRT_DH,
                        )
                    for kc in range(KC):
                        nc.tensor.matmul(
                            dn,
                            ones_t,
                            expT[:, kc, :],
                            start=(kc == 0),
                            stop=(kc == KC - 1),
                        )
                    dr = denp.tile([1, 512], F32, tag="dr")
                    nc.any.tensor_copy(dr, dn)
                    db = denp.tile([128, 512], F32, tag="db")
                    nc.gpsimd.partition_broadcast(db, dr)
                    rb = denp.tile([128, 512], F32, tag="rb")
                    nc.vector.reciprocal(rb, db)
                    return expT, rb

                def pv_phase(qn, expT, rb):
                    # PV: ctxT[d, q], normalized on evacuation
                    q_sl = slice(qn * 512, (qn + 1) * 512)
                    for dc in range(HC):
                        pv = psum_v.tile([128, 512], F32, tag="pv")
                        for kc in range(0, KC, 2):
                            nc.tensor.matmul(
                                pv,
                                v_sb[:, kc:kc + 2, dc * 128:(dc + 1) * 128],
                                expT[:, kc:kc + 2, :],
                                start=(kc == 0),
                                stop=(kc == KC - 2),
                                perf_mode=DR,
                            )
                        nc.any.tensor_mul(ctxT[:, dc, q_sl], pv, rb)

                def epilogue(qn):
                    # out-proj + residual + LN for this q-chunk
                    for sc in range(qn * 4, qn * 4 + 4):
                        x_t = epi.tile([128, H], F32, tag="x")
                        for hn in range(2):
                            ao = psum_v.tile([128, 512], F32, tag="pv")
                            for dc in range(HC):
                                nc.tensor.matmul(
                                    ao,
                                    ctxT[:, dc, sc * 128:(sc + 1) * 128],
                                    wo_sb[:, dc, hn * 512:(hn + 1) * 512],
                                    start=(dc == 0),
                                    stop=(dc == HC - 1),
                                )
                            h_sl = slice(hn * 512, (hn + 1) * 512)
                            # evacuate on ACT (has slack during the epilogue)
                            nc.scalar.activation(
                                x_t[:, h_sl], ao,
                                mybir.ActivationFunctionType.Copy,
                            )
                            # residual add via accumulating DMA (frees DVE)
                            nc.gpsimd.dma_start(
                                x_t[:, h_sl],
                                hsq[sc * 128:(sc + 1) * 128, h_sl],
                                accum_op=mybir.AluOpType.add,
                            )

                        # LayerNorm over h (free dim)
                        st = stat.tile([128, 2, 6], F32, tag="st")
                        nc.vector.bn_stats(st[:, 0, :], x_t[:, 0:512])
                        nc.vector.bn_stats(st[:, 1, :], x_t[:, 512:1024])
                        mv = stat.tile([128, 2], F32, tag="mv")
                        nc.vector.bn_aggr(mv, st)
                        sd = stat.tile([128, 1], F32, tag="sd")
                        nc.scalar.activation(
                            sd, mv[:, 1:2],
                            mybir.ActivationFunctionType.Sqrt, bias=eps_t,
                        )
                        rq = stat.tile([128, 1], F32, tag="rq")
                        nc.vector.reciprocal(rq, sd)
                        y_t = epi.tile([128, H], F32, tag="hsq")
                        nc.vector.tensor_scalar(
                            y_t, x_t, mv[:, 0:1], rq,
                            mybir.AluOpType.subtract, mybir.AluOpType.mult,
                        )
                        if ln_affine:
                            nc.any.tensor_mul(y_t, y_t, g_b)
                            nc.any.tensor_add(y_t, y_t, b_b)
                        nc.sync.dma_start(out_d[sc * 128:(sc + 1) * 128, :], y_t)

                # interleave: epilogue(0) fills the PE shadow between
                # scores(1) and PV(1); den/exp chains hide under matmuls.
                exp0, rb0 = scores_phase(0)
                pv_phase(0, exp0, rb0)
                exp1, rb1 = scores_phase(1)
                epilogue(0)
                pv_phase(1, exp1, rb1)
                epilogue(1)

    nc.compile()
    return nc


def get_program(ln_affine=True):
    key = ("nc", ln_affine)
    if key not in _CACHE:
        _CACHE[key] = _build_program(ln_affine)
    return _CACHE[key]


def make_in_maps(inputs):
    """Host-side sharding / layout prep (numpy only)."""
    f32 = np.float32
    bf16 = ml_dtypes.bfloat16
    hs = np.asarray(inputs["hidden_states"], dtype=f32)
    rel = np.asarray(inputs["word_word_relation"])
    am = np.asarray(inputs["attention_mask"], dtype=f32)  # [B,1,1,S]
    Wq = np.asarray(inputs["Wq"], dtype=f32)
    Wk = np.asarray(inputs["Wk"], dtype=f32)
    Wv = np.asarray(inputs["Wv"], dtype=f32)
    Wo = np.asarray(inputs["Wo"], dtype=f32)
    bq = np.asarray(inputs["bq"], dtype=f32)
    bk = np.asarray(inputs["bk"], dtype=f32)
    bv = np.asarray(inputs["bv"], dtype=f32)
    bo = np.asarray(inputs["bo"], dtype=f32)
    d1 = np.asarray(inputs["dist_emb"], dtype=f32)[1]
    lng = np.asarray(inputs["ln_g"], dtype=f32)
    lnb = np.asarray(inputs["ln_b"], dtype=f32)

    f8 = ml_dtypes.float8_e4m3
    wqT = np.ascontiguousarray(Wq.T * 8.0).astype(f8)
    wkT = np.ascontiguousarray(Wk.T * 8.0).astype(f8)
    wvT = np.ascontiguousarray(Wv.T * 8.0).astype(f8)
    woT = np.ascontiguousarray(Wo.T).astype(bf16)
    bo_eff = Wo @ bv + bo  # v/o biases fold into the residual
    bq_t = np.ascontiguousarray(bq.reshape(HC, 128).T)
    bk_t = np.ascontiguousarray(bk.reshape(HC, 128).T)
    d1_t = np.ascontiguousarray(d1.reshape(HC, 128).T * 8.0).astype(f8)
    rel8 = rel.astype(np.int8)

    in_maps = []
    for c in range(N_CORES):
        b, qh = divmod(c, 2)
        qs = qh * SQ
        in_maps.append({
            "hsT": hs[b].T.astype(f8),
            "hsqT": hs[b, qs:qs + SQ, :].T.astype(f8),
            "hsq": hs[b, qs:qs + SQ, :] + bo_eff,
            "relT": np.ascontiguousarray(rel8[b, qs:qs + SQ, :].T),
            "wqT": wqT, "wkT": wkT, "wvT": wvT, "woT": woT,
            "bq": bq_t, "bk": bk_t, "d1": d1_t,
            "am": np.ascontiguousarray(am[b, 0, 0].reshape(KC, 128).T),
            "lng": lng, "lnb": lnb,
        })
    return in_maps


def kernel(**inputs):
    ln_affine = not (
        np.all(np.asarray(inputs["ln_g"]) == 1.0)
        and np.all(np.asarray(inputs["ln_b"]) == 0.0)
    )
    nc = get_program(ln_affine)
    in_maps = make_in_maps(inputs)
    res = run_bass_kernel_spmd(nc, in_maps, core_ids=list(range(N_CORES)))
    out = np.empty((B, S, H), dtype=np.float32)
    for c in range(N_CORES):
        b, qh = divmod(c, 2)
        out[b, qh * SQ:(qh + 1) * SQ, :] = res.results[c]["out"]
    return out
